# revision 1
# baseline (speedup 1.0000x reference)
"""Trainium2 Bass kernel for nn_AggregatorL1 (GNN message passing).

    self_out  = emb[x0[b]] @ W_self.T  + b_self
    neigh_out = mean_j(emb[x1[b, j]]) @ W_neigh.T + b_neigh
    out[b]    = relu(concat([self_out, neigh_out]))

Distribution: data-parallel over the batch across 8 NeuronCores (2048
nodes per core); embedding table and weights replicated.

Per-core dataflow:
  * All embedding rows are fetched with the SWDGE `dma_gather`
    instruction (hardware-accelerated descriptor generation, one
    descriptor per row, spread over 4 SWDGE queues). Its indices are
    int16, so the 100000-row table is addressed through 4 windows of
    32768 rows; the host splits each core's (node, index) pairs into 4
    per-window streams sorted by node block (index-side sharding prep:
    embedding data is only ever touched on the device).
  * Neighbor rows are gathered from a bf16 copy of the table (they are
    mean-pooled over 32 and carry ~5.7x less magnitude than the self
    features, so the quantization error on the final output is ~2e-4);
    self rows are gathered in full fp32.
  * Gathered rows land position-major: position i -> out[i%128, i//128].
    Rows are reduced to per-node sums with TensorE matmuls whose
    stationary operand is a selection matrix A[p, m] = (tag[p] == m),
    built on-device from host-provided per-position node tags via a
    broadcast is_equal; pad positions carry tag 255 so they contribute
    zero (accumulation in PSUM is fp32 throughout).
  * Per 128-node block: PSUM holds the neighbor sum / self rows
    [128 nodes, 256 feat]; TensorE transposes to feature-major, PE
    projects with W_self/W_neigh (the 1/32 mean is folded into the
    ScalarE activation scale), ScalarE applies bias+relu, TensorE
    transposes back, and the [128, 512] block is DMA'd out.
"""

import os
import sys

sys.path.insert(0, "/opt/trn_rl_repo")

from contextlib import ExitStack

import ml_dtypes
import numpy as np

import concourse.bacc as bacc
import concourse.bass as bass
import concourse.mybir as mybir
import concourse.tile as tile
from concourse import library_config
from concourse.bass_utils import run_bass_kernel_spmd
from concourse.masks import make_identity

N_CORES = 8
B = 16384
NNEIGH = 32
F = 256
H = 256
V = 100000
BPC = B // N_CORES  # 2048 nodes per core
NBLK = BPC // 128  # 16 blocks of 128 nodes
NCHUNK = 4
CW = 32768  # vocab window width (int16 gather index range)
F32 = mybir.dt.float32
BF16 = mybir.dt.bfloat16
I16 = mybir.dt.int16

KSPLIT = int(os.environ.get("KSPLIT", "1"))  # split each neighbor gather N ways
KQMODE = os.environ.get("KQMODE", "win")  # "win": queue=vocab window; "rr": round robin
QBATCH = int(os.environ.get("KQBATCH", "0"))  # batch a window's quarter into one gather if its slot span <= this

_BUILT = {}


def _wrap16(stream_idx):
    """dma_gather index layout: wrapped[p, j] = stream[16*j + p]."""
    return np.ascontiguousarray(stream_idx.reshape(-1, 16).T)


def _host_prep(x0, x1):
    """Split every core's (node, vocab-index) pairs into 4 vocab-window
    streams (neighbors and self separately), sorted by node block, with a
    shared (core-independent) slot/column structure so one SPMD program
    serves all cores.
    """
    x0 = np.asarray(x0, dtype=np.int64)
    x1 = np.asarray(x1, dtype=np.int64)

    per_core = []
    for c in range(N_CORES):
        sl = slice(c * BPC, (c + 1) * BPC)
        n_nei = np.repeat(np.arange(BPC), NNEIGH)
        v_nei = x1[sl].reshape(-1)
        per_core.append(
            {
                "vs": x0[sl],
                "ns": np.arange(BPC),
                "vn": v_nei,
                "nn": n_nei,
            }
        )

    # neighbor segment sizes per (core, q, blk) and self counts
    sizes_n = np.zeros((N_CORES, NCHUNK, NBLK), np.int64)
    sizes_s = np.zeros((N_CORES, NCHUNK, NBLK), np.int64)
    for c in range(N_CORES):
        pc = per_core[c]
        np.add.at(sizes_n, (c, pc["vn"] >> 15, pc["nn"] >> 7), 1)
        np.add.at(sizes_s, (c, pc["vs"] >> 15, pc["ns"] >> 7), 1)
    assert (sizes_s <= 128).all(), "self entries must fit one slot"
    slots = np.maximum(1, -(-sizes_n.max(axis=0) // 128))  # (q, blk) ceil
    seg_start = np.zeros((NCHUNK, NBLK + 1), np.int64)
    for q in range(NCHUNK):
        seg_start[q, 1:] = np.cumsum(slots[q])
    stream_slots = seg_start[:, -1]

    # shared column list: per block, 4 self columns then all neighbor columns
    cols = []  # (q, blk, s, is_self)
    for blk in range(NBLK):
        for q in range(NCHUNK):
            cols.append((q, blk, 0, True))
        for q in range(NCHUNK):
            for s in range(slots[q][blk]):
                cols.append((q, blk, s, False))
    CA = int(slots.sum())  # neighbor columns
    CB = NCHUNK * NBLK  # self columns

    structure = {
        "slots": slots,
        "seg_start": seg_start,
        "stream_slots": stream_slots,
        "cols": cols,
        "CA": CA,
        "CB": CB,
    }

    per_core_arrays = []
    for c in range(N_CORES):
        pc = per_core[c]
        arrs = {"idxn": [], "idxs": []}
        tagsA_streams = []
        tagsB_streams = []
        for qq in range(NCHUNK):
            # neighbor stream
            L = int(stream_slots[qq]) * 128
            stream_idx = np.zeros(L, np.int16)
            tA = np.full(L, 255.0, np.float32)
            sel = np.where((pc["vn"] >> 15) == qq)[0]
            if sel.size:
                bs = pc["nn"][sel] >> 7  # already sorted by node (x1 order)
                seg_first = np.searchsorted(bs, np.arange(NBLK), side="left")
                rank = np.arange(sel.size) - seg_first[bs]
                dest = 128 * seg_start[qq][bs] + rank
                assert (rank < 128 * slots[qq][bs]).all()
                stream_idx[dest] = (pc["vn"][sel] - CW * qq).astype(np.int16)
                tA[dest] = (pc["nn"][sel] & 127).astype(np.float32)
            w16 = _wrap16(stream_idx)
            quarters = []
            for r in range(4):
                c0 = 8 * int(seg_start[qq][4 * r])
                c1 = 8 * int(seg_start[qq][4 * (r + 1)])
                quarters.append(np.ascontiguousarray(np.tile(w16[:, c0:c1], (8, 1))))
            arrs["idxn"].append(quarters)
            tagsA_streams.append(tA)

            # self stream: 16 slots, slot b = block b's self entries
            Ls = NBLK * 128
            s_idx = np.zeros(Ls, np.int16)
            tB = np.full(Ls, 255.0, np.float32)
            sel = np.where((pc["vs"] >> 15) == qq)[0]
            if sel.size:
                bs = sel >> 7  # node id == position; sorted
                seg_first = np.searchsorted(bs, np.arange(NBLK), side="left")
                rank = np.arange(sel.size) - seg_first[bs]
                dest = 128 * bs + rank
                s_idx[dest] = (pc["vs"][sel] - CW * qq).astype(np.int16)
                tB[dest] = (sel & 127).astype(np.float32)
            arrs["idxs"].append(np.ascontiguousarray(np.tile(_wrap16(s_idx), (8, 1))))
            tagsB_streams.append(tB)

        # neighbor tag matrix in column-emission order (bf16)
        tagsA = np.empty((128, CA), np.float32)
        ci = 0
        for qq, bb, s, is_self in cols:
            if is_self:
                continue
            base = 128 * (seg_start[qq][bb] + s)
            tagsA[:, ci] = tagsA_streams[qq][base : base + 128]
            ci += 1
        assert ci == CA
        # self tag matrix: block-major, q inner (matches emission order)
        tagsB = np.empty((128, CB), np.float32)
        ci = 0
        for bb in range(NBLK):
            for qq in range(NCHUNK):
                tagsB[:, ci] = tagsB_streams[qq][128 * bb : 128 * (bb + 1)]
                ci += 1
        arrs["tagsA"] = np.ascontiguousarray(tagsA.astype(ml_dtypes.bfloat16))
        arrs["tagsB"] = tagsB
        per_core_arrays.append(arrs)

    return structure, per_core_arrays


def _build(structure):
    slots = structure["slots"]
    seg_start = structure["seg_start"]
    stream_slots = structure["stream_slots"]
    cols = structure["cols"]
    CA, CB = structure["CA"], structure["CB"]
    SLOTMAX = int(slots.max())
    ATILES = -(-CA // 16)
    BTILES = -(-CB // 16)
    SELF_TB = int(os.environ.get("KSELFTB", "4"))  # blocks of self rows per gather tile

    nc = bacc.Bacc(None, target_bir_lowering=False, debug=True, num_swdge_queues=4)

    emb = nc.dram_tensor("emb", [V, F], F32, kind="ExternalInput")
    emb16 = nc.dram_tensor("emb16", [V, F], BF16, kind="ExternalInput")
    wst = nc.dram_tensor("wst", [F, H], F32, kind="ExternalInput")  # W_self.T
    wnt = nc.dram_tensor("wnt", [F, H], F32, kind="ExternalInput")  # W_neigh.T
    bsd = nc.dram_tensor("bs", [H, 1], F32, kind="ExternalInput")
    bnd = nc.dram_tensor("bn", [H, 1], F32, kind="ExternalInput")
    iota_d = nc.dram_tensor("iota", [128, 16 * 128], F32, kind="ExternalInput")
    iota16_d = nc.dram_tensor("iota16", [128, 16 * 128], BF16, kind="ExternalInput")
    tagsA_d = nc.dram_tensor("tagsA", [128, CA], BF16, kind="ExternalInput")
    tagsB_d = nc.dram_tensor("tagsB", [128, CB], F32, kind="ExternalInput")
    idxn_d = [
        [
            nc.dram_tensor(
                f"idxn{q}_{r}",
                [128, 8 * int(seg_start[q][4 * (r + 1)] - seg_start[q][4 * r])],
                I16,
                kind="ExternalInput",
            )
            for r in range(4)
        ]
        for q in range(NCHUNK)
    ]
    idxs_d = [
        nc.dram_tensor(f"idxs{q}", [128, NBLK * 8], I16, kind="ExternalInput")
        for q in range(NCHUNK)
    ]
    out = nc.dram_tensor("out", [BPC, 2 * H], F32, kind="ExternalOutput")

    with tile.TileContext(nc) as tc, ExitStack() as ctx:
        const = ctx.enter_context(tc.tile_pool(name="const", bufs=1))
        gpool = ctx.enter_context(tc.tile_pool(name="g", bufs=10))
        spool_g = ctx.enter_context(tc.tile_pool(name="gs", bufs=8))
        apool = ctx.enter_context(tc.tile_pool(name="a", bufs=4))
        bpool = ctx.enter_context(tc.tile_pool(name="ab", bufs=2))
        mpool = ctx.enter_context(tc.tile_pool(name="m", bufs=4))
        spool = ctx.enter_context(tc.tile_pool(name="small", bufs=2))
        opool = ctx.enter_context(tc.tile_pool(name="ostage", bufs=2))
        ps_s = ctx.enter_context(tc.tile_pool(name="ps_s", bufs=2, space="PSUM"))
        ps_n = ctx.enter_context(tc.tile_pool(name="ps_n", bufs=2, space="PSUM"))
        ps_t = ctx.enter_context(tc.tile_pool(name="ps_t", bufs=2, space="PSUM"))
        ps_p = ctx.enter_context(tc.tile_pool(name="ps_p", bufs=2, space="PSUM"))

        nc.gpsimd.load_library(library_config.mlp)

        ident = const.tile([128, 128], F32)
        make_identity(nc, ident[:])

        wt = {}
        for path, dram in (("s", wst), ("n", wnt)):
            for k in range(2):
                t = const.tile([128, H], F32, tag=f"w{path}{k}")
                nc.sync.dma_start(out=t[:], in_=dram[128 * k : 128 * (k + 1), :])
                wt[path, k] = t
        bt = {}
        for path, dram in (("s", bsd), ("n", bnd)):
            for h in range(2):
                t = const.tile([128, 1], F32, tag=f"b{path}{h}")
                nc.sync.dma_start(out=t[:], in_=dram[128 * h : 128 * (h + 1), :])
                bt[path, h] = t

        iota_t = const.tile([128, 16 * 128], F32)
        nc.sync.dma_start(out=iota_t[:], in_=iota_d[:])
        iota3d = iota_t[:].rearrange("p (a b) -> p a b", b=128)
        iota16_t = const.tile([128, 16 * 128], BF16)
        nc.sync.dma_start(out=iota16_t[:], in_=iota16_d[:])
        iota16_3d = iota16_t[:].rearrange("p (a b) -> p a b", b=128)

        tagsA_t = const.tile([128, CA], BF16)
        nc.sync.dma_start(out=tagsA_t[:], in_=tagsA_d[:])
        tagsB_t = const.tile([128, CB], F32)
        nc.sync.dma_start(out=tagsB_t[:], in_=tagsB_d[:])

        # index streams, pre-replicated across partition groups by the
        # host and quartered by block range so early gathers only wait on
        # their own quarter's load
        def load_idx(dram, tag):
            t = const.tile([128, dram.shape[1]], I16, tag=tag)
            nc.sync.dma_start(out=t[:], in_=dram[:, :])
            return t

        idxn_t = [
            [load_idx(idxn_d[q][r], f"idxn{q}_{r}") for r in range(4)]
            for q in range(NCHUNK)
        ]
        idxs_t = [load_idx(idxs_d[q], f"idxs{q}") for q in range(NCHUNK)]

        nrep = int(os.environ.get("KREPEAT", "1"))  # perf probing only
        qctr = [0]

        def pick_q(q):
            if KQMODE == "rr":
                q = qctr[0] % NCHUNK
            qctr[0] += 1
            return q

        for _rep in range(nrep):
          gs_tiles = {}  # (q, t) -> tile [128, SELF_TB, F]
          gq_tiles = {}  # (q, quarter) -> batched small-window tile

          def emit_self_gathers(t):
            for q in range(NCHUNK):
                g = spool_g.tile([128, SELF_TB, F], F32, tag="gs")
                nc.gpsimd.dma_gather(
                    g[:],
                    emb[CW * q :, :],
                    idxs_t[q][:, 8 * SELF_TB * t : 8 * SELF_TB * (t + 1)],
                    SELF_TB * 128,
                    SELF_TB * 128,
                    F,
                    single_packet=False,
                    queue_num=pick_q(q),
                )
                gs_tiles[q, t] = g

          # self selection-matrix tiles (fp32)
          b_tiles = []
          for u in range(BTILES):
            lo, hi = 16 * u, min(CB, 16 * u + 16)
            at = bpool.tile([128, 16, 128], F32, tag="ab")
            nc.vector.tensor_tensor(
                out=at[:, 0 : hi - lo, :],
                in0=tagsB_t[:, lo:hi].to_broadcast([128, hi - lo, 128]),
                in1=iota3d[:, 0 : hi - lo, :],
                op=mybir.AluOpType.is_equal,
            )
            b_tiles.append(at)

          # neighbor selection-matrix tiles (bf16), in column order
          a_tiles = []
          for u in range(ATILES):
            lo, hi = 16 * u, min(CA, 16 * u + 16)
            at = apool.tile([128, 16, 128], BF16, tag="a")
            nc.vector.tensor_tensor(
                out=at[:, 0 : hi - lo, :],
                in0=tagsA_t[:, lo:hi].to_broadcast([128, hi - lo, 128]),
                in1=iota16_3d[:, 0 : hi - lo, :],
                op=mybir.AluOpType.is_equal,
            )
            a_tiles.append(at)

          ca = 0  # neighbor column counter
          cb = 0  # self column counter
          for blk in range(NBLK):
            if blk % SELF_TB == 0:
                emit_self_gathers(blk // SELF_TB)
            # gather the 4 neighbor segments of this block (bf16)
            g_tiles = []
            r = blk // 4
            for q in range(NCHUNK):
                ns = int(slots[q][blk])
                s0 = int(seg_start[q][blk])
                b0 = 8 * int(seg_start[q][4 * r])
                span = int(seg_start[q][4 * (r + 1)] - seg_start[q][4 * r])
                spanmax = max(
                    int(seg_start[q][4 * (rr2 + 1)] - seg_start[q][4 * rr2])
                    for rr2 in range(4)
                )
                if span <= QBATCH:
                    # small window: one gather covers the whole 4-block quarter
                    if (q, r) not in gq_tiles:
                        gq = gpool.tile([128, spanmax, F], BF16, tag=f"gq{q}")
                        nc.gpsimd.dma_gather(
                            gq[:, 0:span, :],
                            emb16[CW * q :, :],
                            idxn_t[q][r][:, :],
                            span * 128,
                            span * 128,
                            F,
                            single_packet=False,
                            queue_num=pick_q(q),
                        )
                        gq_tiles[q, r] = gq
                    rel = s0 - int(seg_start[q][4 * r])
                    g_tiles.append((gq_tiles[q, r], rel))
                    continue
                g = gpool.tile([128, SLOTMAX, F], BF16, tag="g")
                step = -(-ns // KSPLIT)
                a = 0
                while a < ns:
                    b = min(ns, a + step)
                    nc.gpsimd.dma_gather(
                        g[:, a:b, :],
                        emb16[CW * q :, :],
                        idxn_t[q][r][:, 8 * (s0 + a) - b0 : 8 * (s0 + b) - b0],
                        (b - a) * 128,
                        (b - a) * 128,
                        F,
                        single_packet=False,
                        queue_num=pick_q(q),
                    )
                    a = b
                g_tiles.append((g, 0))

            # self scatter: 4 fp32 columns (slot blk%SELF_TB of tile blk//SELF_TB)
            psum_s = ps_s.tile([128, F], F32, tag="ps")
            for q in range(NCHUNK):
                nc.tensor.matmul(
                    out=psum_s[:],
                    lhsT=b_tiles[cb // 16][:, cb % 16, :],
                    rhs=gs_tiles[q, blk // SELF_TB][:, blk % SELF_TB, :],
                    start=(q == 0),
                    stop=(q == NCHUNK - 1),
                )
                cb += 1
            ms = mpool.tile([128, F], F32, tag="m")
            nc.any.tensor_copy(out=ms[:], in_=psum_s[:])

            # neighbor sum columns (bf16 data, fp32 PSUM accumulation)
            psum_n = ps_n.tile([128, F], F32, tag="pn")
            ncols = int(slots[:, blk].sum())
            done = 0
            for q in range(NCHUNK):
                gt, rel = g_tiles[q]
                for s in range(int(slots[q][blk])):
                    nc.tensor.matmul(
                        out=psum_n[:],
                        lhsT=a_tiles[ca // 16][:, ca % 16, :],
                        rhs=gt[:, rel + s, :],
                        start=(done == 0),
                        stop=(done == ncols - 1),
                    )
                    ca += 1
                    done += 1
            mn = mpool.tile([128, F], F32, tag="m")
            nc.any.tensor_copy(out=mn[:], in_=psum_n[:])

            # downstream: transpose -> project -> bias+relu -> transpose back
            ostage = opool.tile([128, 4 * 128], F32, tag="ostage")
            for path, src in (("s", ms), ("n", mn)):
                fchunks = []
                for k in range(2):
                    pt = ps_t.tile([128, 128], F32, tag="pt")
                    nc.tensor.transpose(
                        out=pt[:],
                        in_=src[:, 128 * k : 128 * (k + 1)],
                        identity=ident[:],
                    )
                    st = spool.tile([128, 128], F32, tag=f"st{k}")
                    nc.any.tensor_copy(out=st[:], in_=pt[:])
                    fchunks.append(st)
                scale = 1.0 if path == "s" else 1.0 / NNEIGH
                for h in range(2):
                    pp = ps_p.tile([128, 128], F32, tag="pp")
                    for k in range(2):
                        nc.tensor.matmul(
                            out=pp[:],
                            lhsT=wt[path, k][:, 128 * h : 128 * (h + 1)],
                            rhs=fchunks[k][:],
                            start=(k == 0),
                            stop=(k == 1),
                        )
                    at2 = spool.tile([128, 128], F32, tag="act")
                    nc.scalar.activation(
                        out=at2[:],
                        in_=pp[:],
                        func=mybir.ActivationFunctionType.Relu,
                        bias=bt[path, h][:],
                        scale=scale,
                    )
                    po = ps_t.tile([128, 128], F32, tag="pt")
                    nc.tensor.transpose(out=po[:], in_=at2[:], identity=ident[:])
                    slot = (0 if path == "s" else 2) + h
                    nc.any.tensor_copy(
                        out=ostage[:, 128 * slot : 128 * (slot + 1)], in_=po[:]
                    )
            nc.sync.dma_start(
                out=out[128 * blk : 128 * (blk + 1), :], in_=ostage[:]
            )
          assert ca == CA and cb == CB

    nc.compile()
    return nc


def _prep_and_build(x0, x1):
    structure, per_core = _host_prep(x0, x1)
    key = (structure["slots"].tobytes(), structure["CA"], os.environ.get("KREPEAT","1"), KSPLIT, KQMODE, os.environ.get("KSELFTB","4"), QBATCH)
    if _BUILT.get("key") != key:
        _BUILT["nc"] = _build(structure)
        _BUILT["key"] = key
    return _BUILT["nc"], structure, per_core


def make_in_maps(x0, x1, emb, W_self, b_self, W_neigh, b_neigh):
    nc, structure, per_core = _prep_and_build(x0, x1)
    emb = np.ascontiguousarray(np.asarray(emb, dtype=np.float32))
    emb16 = np.ascontiguousarray(emb.astype(ml_dtypes.bfloat16))
    wstv = np.ascontiguousarray(np.asarray(W_self, dtype=np.float32).T)
    wntv = np.ascontiguousarray(np.asarray(W_neigh, dtype=np.float32).T)
    bsv = np.ascontiguousarray(np.asarray(b_self, dtype=np.float32).reshape(H, 1))
    bnv = np.ascontiguousarray(np.asarray(b_neigh, dtype=np.float32).reshape(H, 1))
    iota = np.ascontiguousarray(np.tile(np.arange(128, dtype=np.float32), (128, 16)))
    iota16 = np.ascontiguousarray(iota.astype(ml_dtypes.bfloat16))
    in_maps = []
    for c in range(N_CORES):
        m = {
            "emb": emb,
            "emb16": emb16,
            "wst": wstv,
            "wnt": wntv,
            "bs": bsv,
            "bn": bnv,
            "iota": iota,
            "iota16": iota16,
            "tagsA": per_core[c]["tagsA"],
            "tagsB": per_core[c]["tagsB"],
        }
        for q in range(NCHUNK):
            for r in range(4):
                m[f"idxn{q}_{r}"] = per_core[c]["idxn"][q][r]
            m[f"idxs{q}"] = per_core[c]["idxs"][q]
        in_maps.append(m)
    return nc, in_maps


def kernel(x0, x1, emb, W_self, b_self, W_neigh, b_neigh, **_ignored):
    nc, in_maps = make_in_maps(x0, x1, emb, W_self, b_self, W_neigh, b_neigh)
    res = run_bass_kernel_spmd(nc, in_maps, core_ids=list(range(N_CORES)))
    return np.concatenate([r["out"] for r in res.results], axis=0)



# revision 5
# speedup vs baseline: 1.9609x; 1.9609x over previous
"""Trainium2 Bass kernel for nn_AggregatorL1 (GNN message passing).

    self_out  = emb[x0[b]] @ W_self.T  + b_self
    neigh_out = mean_j(emb[x1[b, j]]) @ W_neigh.T + b_neigh
    out[b]    = relu(concat([self_out, neigh_out]))

Distribution: data-parallel over the batch across 8 NeuronCores (2048
nodes per core); embedding table and weights replicated.

Per-core dataflow (all-bf16 datapath; fp32 only in PSUM accumulation):
  * One unified draw stream per 32768-row vocab window (int16 gather
    index range): for each (window, 128-node block) segment, the block's
    SELF draws come first (they fit slot 0), then its neighbor draws,
    padded to 128-position slots. SWDGE `dma_gather` fetches rows from a
    bf16 table copy in a few large gathers (GG blocks per gather, spread
    over 4 SWDGE queues) - position i lands at [i%128, i//128].
  * Per block, rows are reduced to per-node sums with TensorE matmuls in
    FEATURE-major orientation: psum[f_half, node] += G_slot[:, f_half].T
    @ A_slot, where A[p, n] = (tagA[p] == n) is built on-device by DVE
    is_equal from host tags (pad/self positions carry tag 255 -> zero).
    Slot 0 additionally accumulates the self sum via tagB columns.
  * Projection without transposes: out_psum[node, h] += SUM_fm[f_half,
    node].T @ W.T-chunk (weights bf16; the 1/32 neighbor mean is folded
    into W_neigh.T host-side), plus one bias matmul (lhsT = E with row 0
    of ones, rhs = bias broadcast row), then ScalarE relu PSUM -> SBUF
    and one [128, 512] DMA out per block.
"""

import os
import sys

sys.path.insert(0, "/opt/trn_rl_repo")

from contextlib import ExitStack

import ml_dtypes
import numpy as np

import concourse.bacc as bacc
import concourse.bass as bass
import concourse.mybir as mybir
import concourse.tile as tile
from concourse import library_config
from concourse.bass_utils import run_bass_kernel_spmd

N_CORES = 8
B = 16384
NNEIGH = 32
F = 256
H = 256
V = 100000
BPC = B // N_CORES  # 2048 nodes per core
NBLK = BPC // 128  # 16 blocks of 128 nodes
NQ = 4
CW = 32768  # vocab window width (int16 gather index range)
F32 = mybir.dt.float32
BF16 = mybir.dt.bfloat16
I16 = mybir.dt.int16

GG = int(os.environ.get("KGG", "2"))  # blocks per gather group
KQMODE = os.environ.get("KQMODE", "rr")  # gather queue pick: rr | win

_BUILT = {}


def _wrap16(stream_idx):
    """dma_gather index layout: wrapped[p, j] = stream[16*j + p]."""
    return np.ascontiguousarray(stream_idx.reshape(-1, 16).T)


def _host_prep(x0, x1):
    """Build the per-window draw streams (self-first segments per (q, blk)),
    with a core-independent slot structure so one SPMD program serves all
    cores; emit int16 index streams and bf16 tag matrices per core."""
    x0 = np.asarray(x0, dtype=np.int64)
    x1 = np.asarray(x1, dtype=np.int64)

    per_core = []
    for c in range(N_CORES):
        sl = slice(c * BPC, (c + 1) * BPC)
        per_core.append(
            {
                "vs": x0[sl],  # [BPC]
                "vn": x1[sl].reshape(-1),  # [BPC*NNEIGH], node-major
                "nn": np.repeat(np.arange(BPC), NNEIGH),
            }
        )

    # per (core, q, blk): self + neighbor counts
    sizes = np.zeros((N_CORES, NQ, NBLK), np.int64)
    sizes_s = np.zeros((N_CORES, NQ, NBLK), np.int64)
    for c in range(N_CORES):
        pc = per_core[c]
        np.add.at(sizes, (c, pc["vn"] >> 15, pc["nn"] >> 7), 1)
        np.add.at(sizes_s, (c, pc["vs"] >> 15, np.arange(BPC) >> 7), 1)
    sizes += sizes_s
    assert (sizes_s <= 128).all(), "self entries must fit in slot 0"
    slots = np.maximum(1, -(-sizes.max(axis=0) // 128))  # (q, blk)
    seg_start = np.zeros((NQ, NBLK + 1), np.int64)
    for q in range(NQ):
        seg_start[q, 1:] = np.cumsum(slots[q])

    CA = int(slots.sum())  # A columns: one per slot
    CB = NQ * NBLK  # B columns: slot 0 of each (q, blk)

    structure = {"slots": slots, "seg_start": seg_start, "CA": CA, "CB": CB}

    per_core_arrays = []
    for c in range(N_CORES):
        pc = per_core[c]
        arrs = {"idx": []}
        tagsA_streams = []
        tagsB_streams = []
        for q in range(NQ):
            L = int(seg_start[q][-1]) * 128
            stream_idx = np.zeros(L, np.int16)
            tA = np.full(L, 255.0, np.float32)
            tB = np.full(L, 255.0, np.float32)
            # self draws first within each (q, blk) segment
            sel_s = np.where((pc["vs"] >> 15) == q)[0]  # node ids
            bs_s = sel_s >> 7
            seg_first = np.searchsorted(bs_s, np.arange(NBLK), side="left")
            rank = np.arange(sel_s.size) - seg_first[bs_s]
            dest_s = 128 * seg_start[q][bs_s] + rank
            stream_idx[dest_s] = (pc["vs"][sel_s] - CW * q).astype(np.int16)
            tB[dest_s] = (sel_s & 127).astype(np.float32)
            n_self_blk = np.zeros(NBLK, np.int64)
            np.add.at(n_self_blk, bs_s, 1)
            # neighbor draws after the block's self draws
            sel_n = np.where((pc["vn"] >> 15) == q)[0]
            bs_n = pc["nn"][sel_n] >> 7  # node-major -> sorted by block
            seg_first = np.searchsorted(bs_n, np.arange(NBLK), side="left")
            rank = np.arange(sel_n.size) - seg_first[bs_n]
            dest_n = 128 * seg_start[q][bs_n] + n_self_blk[bs_n] + rank
            assert (rank + n_self_blk[bs_n] < 128 * slots[q][bs_n]).all()
            stream_idx[dest_n] = (pc["vn"][sel_n] - CW * q).astype(np.int16)
            tA[dest_n] = (pc["nn"][sel_n] & 127).astype(np.float32)

            w16 = _wrap16(stream_idx)
            groups = []
            for r in range(-(-NBLK // GG)):
                c0 = 8 * int(seg_start[q][GG * r])
                c1 = 8 * int(seg_start[q][min(NBLK, GG * (r + 1))])
                groups.append(np.ascontiguousarray(np.tile(w16[:, c0:c1], (8, 1))))
            arrs["idx"].append(groups)
            tagsA_streams.append(tA)
            tagsB_streams.append(tB)

        # A columns in consumption order: for blk: for q: for s
        tagsA = np.empty((128, CA), np.float32)
        tagsB = np.empty((128, CB), np.float32)
        ca = cb = 0
        for blk in range(NBLK):
            for q in range(NQ):
                base = 128 * seg_start[q][blk]
                for s in range(int(slots[q][blk])):
                    tagsA[:, ca] = tagsA_streams[q][base + 128 * s : base + 128 * (s + 1)]
                    ca += 1
                tagsB[:, cb] = tagsB_streams[q][base : base + 128]
                cb += 1
        assert ca == CA and cb == CB
        arrs["tagsA"] = np.ascontiguousarray(tagsA.astype(ml_dtypes.bfloat16))
        arrs["tagsB"] = np.ascontiguousarray(tagsB.astype(ml_dtypes.bfloat16))
        per_core_arrays.append(arrs)

    return structure, per_core_arrays


def _build(structure):
    slots = structure["slots"]
    seg_start = structure["seg_start"]
    CA, CB = structure["CA"], structure["CB"]
    NR = -(-NBLK // GG)  # gather groups per window
    ATILES = -(-CA // 16)
    BTILES = -(-CB // 16)

    nc = bacc.Bacc(None, target_bir_lowering=False, debug=True, num_swdge_queues=4)

    emb16 = nc.dram_tensor("emb16", [V, F], BF16, kind="ExternalInput")
    wst = nc.dram_tensor("wst", [F, H], BF16, kind="ExternalInput")  # W_self.T
    wnt = nc.dram_tensor("wnt", [F, H], BF16, kind="ExternalInput")  # W_neigh.T/32
    brow_d = nc.dram_tensor("brow", [128, 2 * H], BF16, kind="ExternalInput")
    eone_d = nc.dram_tensor("eone", [128, 128], BF16, kind="ExternalInput")
    iota16_d = nc.dram_tensor("iota16", [128, 16 * 128], BF16, kind="ExternalInput")
    tagsA_d = nc.dram_tensor("tagsA", [128, CA], BF16, kind="ExternalInput")
    tagsB_d = nc.dram_tensor("tagsB", [128, CB], BF16, kind="ExternalInput")
    idx_d = [
        [
            nc.dram_tensor(
                f"idx{q}_{r}",
                [
                    128,
                    8
                    * int(
                        seg_start[q][min(NBLK, GG * (r + 1))] - seg_start[q][GG * r]
                    ),
                ],
                I16,
                kind="ExternalInput",
            )
            for r in range(NR)
        ]
        for q in range(NQ)
    ]
    out = nc.dram_tensor("out", [BPC, 2 * H], F32, kind="ExternalOutput")

    SMAX = [
        max(
            int(seg_start[q][min(NBLK, GG * (r + 1))] - seg_start[q][GG * r])
            for r in range(NR)
        )
        for q in range(NQ)
    ]

    with tile.TileContext(nc) as tc, ExitStack() as ctx:
        const = ctx.enter_context(tc.tile_pool(name="const", bufs=1))
        gpools = [
            ctx.enter_context(tc.tile_pool(name=f"g{q}", bufs=2)) for q in range(NQ)
        ]
        apool = ctx.enter_context(tc.tile_pool(name="a", bufs=4))
        bpool = ctx.enter_context(tc.tile_pool(name="b", bufs=2))
        sumpool = ctx.enter_context(tc.tile_pool(name="sum", bufs=4))
        opool = ctx.enter_context(tc.tile_pool(name="ostage", bufs=2))
        ps_sel = ctx.enter_context(tc.tile_pool(name="ps_sel", bufs=2, space="PSUM"))
        ps_out = ctx.enter_context(tc.tile_pool(name="ps_out", bufs=2, space="PSUM"))

        nc.gpsimd.load_library(library_config.mlp)

        wt = {}
        for path, dram in (("s", wst), ("n", wnt)):
            for k in range(2):
                t = const.tile([128, H], BF16, tag=f"w{path}{k}")
                nc.sync.dma_start(out=t[:], in_=dram[128 * k : 128 * (k + 1), :])
                wt[path, k] = t
        brow_t = const.tile([128, 2 * H], BF16)
        nc.sync.dma_start(out=brow_t[:], in_=brow_d[:])
        eone_t = const.tile([128, 128], BF16)
        nc.sync.dma_start(out=eone_t[:], in_=eone_d[:])
        iota16_t = const.tile([128, 16 * 128], BF16)
        nc.sync.dma_start(out=iota16_t[:], in_=iota16_d[:])
        iota16_3d = iota16_t[:].rearrange("p (a b) -> p a b", b=128)
        tagsA_t = const.tile([128, CA], BF16)
        nc.sync.dma_start(out=tagsA_t[:], in_=tagsA_d[:])
        tagsB_t = const.tile([128, CB], BF16)
        nc.sync.dma_start(out=tagsB_t[:], in_=tagsB_d[:])
        def load_idx(dram, tag):
            t = const.tile([128, dram.shape[1]], I16, tag=tag)
            nc.sync.dma_start(out=t[:], in_=dram[:, :])
            return t

        idx_t = [
            [load_idx(idx_d[q][r], f"idx{q}_{r}") for r in range(NR)]
            for q in range(NQ)
        ]

        nrep = int(os.environ.get("KREPEAT", "1"))  # perf probing only
        qctr = [0]

        def pick_q(q):
            picked = qctr[0] % NQ if KQMODE == "rr" else q
            qctr[0] += 1
            return picked

        for _rep in range(nrep):
            g_tiles = {}

            def emit_gathers(r):
                for q in range(NQ):
                    span = int(
                        seg_start[q][min(NBLK, GG * (r + 1))] - seg_start[q][GG * r]
                    )
                    g = gpools[q].tile([128, SMAX[q], F], BF16, tag=f"g{q}")
                    nc.gpsimd.dma_gather(
                        g[:, 0:span, :],
                        emb16[CW * q :, :],
                        idx_t[q][r][:, :],
                        span * 128,
                        span * 128,
                        F,
                        single_packet=False,
                        queue_num=pick_q(q),
                    )
                    g_tiles[q, r] = g

            # selection matrices, built in 16-column tiles (pool-throttled)
            a_tiles = []
            for u in range(ATILES):
                lo, hi = 16 * u, min(CA, 16 * u + 16)
                at = apool.tile([128, 16, 128], BF16, tag="a")
                nc.vector.tensor_tensor(
                    out=at[:, 0 : hi - lo, :],
                    in0=tagsA_t[:, lo:hi].to_broadcast([128, hi - lo, 128]),
                    in1=iota16_3d[:, 0 : hi - lo, :],
                    op=mybir.AluOpType.is_equal,
                )
                a_tiles.append(at)
            b_tiles = []
            for u in range(BTILES):
                lo, hi = 16 * u, min(CB, 16 * u + 16)
                bt = bpool.tile([128, 16, 128], BF16, tag="b")
                nc.vector.tensor_tensor(
                    out=bt[:, 0 : hi - lo, :],
                    in0=tagsB_t[:, lo:hi].to_broadcast([128, hi - lo, 128]),
                    in1=iota16_3d[:, 0 : hi - lo, :],
                    op=mybir.AluOpType.is_equal,
                )
                b_tiles.append(bt)

            ca = cb = 0
            for blk in range(NBLK):
                if blk % GG == 0:
                    emit_gathers(blk // GG)
                psn = ps_sel.tile([128, 2 * 128], F32, tag="pn")
                pss = ps_sel.tile([128, 2 * 128], F32, tag="ps")
                ncols = int(slots[:, blk].sum())
                done = 0
                for q in range(NQ):
                    r = blk // GG
                    g = g_tiles[q, r]
                    rel = int(seg_start[q][blk] - seg_start[q][GG * r])
                    for s in range(int(slots[q][blk])):
                        acol = a_tiles[ca // 16][:, ca % 16, :]
                        for fh in range(2):
                            # one start/stop per PSUM bank: start marks the
                            # whole 2KB zero region pending-zero, so only the
                            # first matmul into the bank may carry it
                            lhs = g[:, rel + s, 128 * fh : 128 * (fh + 1)]
                            nc.tensor.matmul(
                                out=psn[:, 128 * fh : 128 * (fh + 1)],
                                lhsT=lhs,
                                rhs=acol,
                                start=(done == 0 and fh == 0),
                                stop=(done == ncols - 1 and fh == 1),
                                skip_group_check=True,
                            )
                            if s == 0:
                                nc.tensor.matmul(
                                    out=pss[:, 128 * fh : 128 * (fh + 1)],
                                    lhsT=lhs,
                                    rhs=b_tiles[cb // 16][:, cb % 16, :],
                                    start=(q == 0 and fh == 0),
                                    stop=(q == NQ - 1 and fh == 1),
                                    skip_group_check=True,
                                )
                        ca += 1
                        done += 1
                    cb += 1

                ostage = opool.tile([128, 4 * 128], F32, tag="ostage")
                for pi, (path, ps) in enumerate((("s", pss), ("n", psn))):
                    sumt = sumpool.tile([128, 2 * 128], BF16, tag="sum")
                    # ScalarE copy: DVE is loaded with is_equal builds, ACT idle
                    nc.scalar.activation(
                        out=sumt[:],
                        in_=ps[:],
                        func=mybir.ActivationFunctionType.Copy,
                    )
                    po = ps_out.tile([128, 2 * 128], F32, tag=f"po{path}")
                    for k in range(2):
                        nc.tensor.matmul(
                            out=po[:],
                            lhsT=sumt[:, 128 * k : 128 * (k + 1)],
                            rhs=wt[path, k][:],
                            start=(k == 0),
                            stop=False,
                        )
                    nc.tensor.matmul(
                        out=po[:],
                        lhsT=eone_t[:],
                        rhs=brow_t[:, 256 * pi : 256 * (pi + 1)],
                        start=False,
                        stop=True,
                    )
                    nc.scalar.activation(
                        out=ostage[:, 256 * pi : 256 * (pi + 1)],
                        in_=po[:],
                        func=mybir.ActivationFunctionType.Relu,
                    )
                nc.sync.dma_start(
                    out=out[128 * blk : 128 * (blk + 1), :], in_=ostage[:]
                )
            assert ca == CA and cb == CB

    nc.compile()
    return nc


def _prep_and_build(x0, x1):
    structure, per_core = _host_prep(x0, x1)
    key = (
        structure["slots"].tobytes(),
        structure["CA"],
        os.environ.get("KREPEAT", "1"),
        GG,
        KQMODE,
    )
    if _BUILT.get("key") != key:
        _BUILT["nc"] = _build(structure)
        _BUILT["key"] = key
    return _BUILT["nc"], structure, per_core


def make_in_maps(x0, x1, emb, W_self, b_self, W_neigh, b_neigh):
    nc, structure, per_core = _prep_and_build(x0, x1)
    emb16 = np.ascontiguousarray(
        np.asarray(emb, dtype=np.float32).astype(ml_dtypes.bfloat16)
    )
    wstv = np.ascontiguousarray(
        np.asarray(W_self, dtype=np.float32).T.astype(ml_dtypes.bfloat16)
    )
    wntv = np.ascontiguousarray(
        (np.asarray(W_neigh, dtype=np.float32).T / NNEIGH).astype(ml_dtypes.bfloat16)
    )
    brow = np.zeros((128, 2 * H), np.float32)
    brow[0, :H] = np.asarray(b_self, dtype=np.float32)
    brow[0, H:] = np.asarray(b_neigh, dtype=np.float32)
    brow = np.ascontiguousarray(brow.astype(ml_dtypes.bfloat16))
    eone = np.zeros((128, 128), np.float32)
    eone[0, :] = 1.0
    eone = np.ascontiguousarray(eone.astype(ml_dtypes.bfloat16))
    iota16 = np.ascontiguousarray(
        np.tile(np.arange(128, dtype=np.float32), (128, 16)).astype(ml_dtypes.bfloat16)
    )
    NR = -(-NBLK // GG)
    in_maps = []
    for c in range(N_CORES):
        m = {
            "emb16": emb16,
            "wst": wstv,
            "wnt": wntv,
            "brow": brow,
            "eone": eone,
            "iota16": iota16,
            "tagsA": per_core[c]["tagsA"],
            "tagsB": per_core[c]["tagsB"],
        }
        for q in range(NQ):
            for r in range(NR):
                m[f"idx{q}_{r}"] = per_core[c]["idx"][q][r]
        in_maps.append(m)
    return nc, in_maps


def kernel(x0, x1, emb, W_self, b_self, W_neigh, b_neigh, **_ignored):
    nc, in_maps = make_in_maps(x0, x1, emb, W_self, b_self, W_neigh, b_neigh)
    res = run_bass_kernel_spmd(nc, in_maps, core_ids=list(range(N_CORES)))
    return np.concatenate([r["out"] for r in res.results], axis=0)


# revision 7
# speedup vs baseline: 2.1209x; 1.0816x over previous
"""Trainium2 Bass kernel for nn_AggregatorL1 (GNN message passing).

    self_out  = emb[x0[b]] @ W_self.T  + b_self
    neigh_out = mean_j(emb[x1[b, j]]) @ W_neigh.T + b_neigh
    out[b]    = relu(concat([self_out, neigh_out]))

Distribution: data-parallel over the batch across 8 NeuronCores (2048
nodes per core); embedding table and weights replicated.

Per-core dataflow (all-bf16 datapath; fp32 only in PSUM accumulation):
  * One unified draw stream per 32768-row vocab window (int16 gather
    index range): for each (window, 128-node block) segment, the block's
    SELF draws come first (they fit slot 0), then its neighbor draws,
    padded to 128-position slots. SWDGE `dma_gather` fetches rows from a
    bf16 table copy in a few large gathers (GG blocks per gather, spread
    over 4 SWDGE queues) - position i lands at [i%128, i//128].
  * Per block, rows are reduced to per-node sums with TensorE matmuls in
    FEATURE-major orientation: psum[f_half, node] += G_slot[:, f_half].T
    @ A_slot, where A[p, n] = (tagA[p] == n) is built on-device by DVE
    is_equal from host tags (pad/self positions carry tag 255 -> zero).
    Slot 0 additionally accumulates the self sum via tagB columns.
  * Projection without transposes: out_psum[node, h] += SUM_fm[f_half,
    node].T @ W.T-chunk (weights bf16; the 1/32 neighbor mean is folded
    into W_neigh.T host-side), plus one bias matmul (lhsT = E with row 0
    of ones, rhs = bias broadcast row), then ScalarE relu PSUM -> SBUF
    and one [128, 512] DMA out per block.
"""

import os
import sys

sys.path.insert(0, "/opt/trn_rl_repo")

from contextlib import ExitStack

import ml_dtypes
import numpy as np

import concourse.bacc as bacc
import concourse.bass as bass
import concourse.mybir as mybir
import concourse.tile as tile
from concourse import library_config
from concourse.bass_utils import run_bass_kernel_spmd

N_CORES = 8
B = 16384
NNEIGH = 32
F = 256
H = 256
V = 100000
BPC = B // N_CORES  # 2048 nodes per core
NBLK = BPC // 128  # 16 blocks of 128 nodes
NQ = 4
CW = 32768  # vocab window width (int16 gather index range)
F32 = mybir.dt.float32
BF16 = mybir.dt.bfloat16
I16 = mybir.dt.int16

GG = int(os.environ.get("KGG", "2"))  # blocks per gather group
KQMODE = os.environ.get("KQMODE", "rr")  # gather queue pick: rr | win

_BUILT = {}


def _wrap16(stream_idx):
    """dma_gather index layout: wrapped[p, j] = stream[16*j + p]."""
    return np.ascontiguousarray(stream_idx.reshape(-1, 16).T)


def _host_prep(x0, x1):
    """Build the per-window draw streams (self-first segments per (q, blk)),
    with a core-independent slot structure so one SPMD program serves all
    cores; emit int16 index streams and bf16 tag matrices per core."""
    x0 = np.asarray(x0, dtype=np.int64)
    x1 = np.asarray(x1, dtype=np.int64)

    per_core = []
    for c in range(N_CORES):
        sl = slice(c * BPC, (c + 1) * BPC)
        per_core.append(
            {
                "vs": x0[sl],  # [BPC]
                "vn": x1[sl].reshape(-1),  # [BPC*NNEIGH], node-major
                "nn": np.repeat(np.arange(BPC), NNEIGH),
            }
        )

    # per (core, q, blk): self + neighbor counts
    sizes = np.zeros((N_CORES, NQ, NBLK), np.int64)
    sizes_s = np.zeros((N_CORES, NQ, NBLK), np.int64)
    for c in range(N_CORES):
        pc = per_core[c]
        np.add.at(sizes, (c, pc["vn"] >> 15, pc["nn"] >> 7), 1)
        np.add.at(sizes_s, (c, pc["vs"] >> 15, np.arange(BPC) >> 7), 1)
    sizes += sizes_s
    assert (sizes_s <= 128).all(), "self entries must fit in slot 0"
    slots = np.maximum(1, -(-sizes.max(axis=0) // 128))  # (q, blk)
    seg_start = np.zeros((NQ, NBLK + 1), np.int64)
    for q in range(NQ):
        seg_start[q, 1:] = np.cumsum(slots[q])

    CA = int(slots.sum())  # A columns: one per slot
    CB = NQ * NBLK  # B columns: slot 0 of each (q, blk)

    structure = {"slots": slots, "seg_start": seg_start, "CA": CA, "CB": CB}

    per_core_arrays = []
    for c in range(N_CORES):
        pc = per_core[c]
        arrs = {"idx": []}
        tagsA_streams = []
        tagsB_streams = []
        for q in range(NQ):
            L = int(seg_start[q][-1]) * 128
            stream_idx = np.zeros(L, np.int16)
            tA = np.full(L, 255.0, np.float32)
            tB = np.full(L, 255.0, np.float32)
            # self draws first within each (q, blk) segment
            sel_s = np.where((pc["vs"] >> 15) == q)[0]  # node ids
            bs_s = sel_s >> 7
            seg_first = np.searchsorted(bs_s, np.arange(NBLK), side="left")
            rank = np.arange(sel_s.size) - seg_first[bs_s]
            dest_s = 128 * seg_start[q][bs_s] + rank
            stream_idx[dest_s] = (pc["vs"][sel_s] - CW * q).astype(np.int16)
            tB[dest_s] = (sel_s & 127).astype(np.float32)
            n_self_blk = np.zeros(NBLK, np.int64)
            np.add.at(n_self_blk, bs_s, 1)
            # neighbor draws after the block's self draws
            sel_n = np.where((pc["vn"] >> 15) == q)[0]
            bs_n = pc["nn"][sel_n] >> 7  # node-major -> sorted by block
            seg_first = np.searchsorted(bs_n, np.arange(NBLK), side="left")
            rank = np.arange(sel_n.size) - seg_first[bs_n]
            dest_n = 128 * seg_start[q][bs_n] + n_self_blk[bs_n] + rank
            assert (rank + n_self_blk[bs_n] < 128 * slots[q][bs_n]).all()
            stream_idx[dest_n] = (pc["vn"][sel_n] - CW * q).astype(np.int16)
            tA[dest_n] = (pc["nn"][sel_n] & 127).astype(np.float32)

            w16 = _wrap16(stream_idx)
            groups = []
            for r in range(-(-NBLK // GG)):
                c0 = 8 * int(seg_start[q][GG * r])
                c1 = 8 * int(seg_start[q][min(NBLK, GG * (r + 1))])
                groups.append(np.ascontiguousarray(np.tile(w16[:, c0:c1], (8, 1))))
            arrs["idx"].append(groups)
            tagsA_streams.append(tA)
            tagsB_streams.append(tB)

        # A columns in consumption order: for blk: for q: for s
        tagsA = np.empty((128, CA), np.float32)
        tagsB = np.empty((128, CB), np.float32)
        ca = cb = 0
        for blk in range(NBLK):
            for q in range(NQ):
                base = 128 * seg_start[q][blk]
                for s in range(int(slots[q][blk])):
                    tagsA[:, ca] = tagsA_streams[q][base + 128 * s : base + 128 * (s + 1)]
                    ca += 1
                tagsB[:, cb] = tagsB_streams[q][base : base + 128]
                cb += 1
        assert ca == CA and cb == CB
        arrs["tagsA"] = np.ascontiguousarray(tagsA.astype(ml_dtypes.bfloat16))
        arrs["tagsB"] = np.ascontiguousarray(tagsB.astype(ml_dtypes.bfloat16))
        per_core_arrays.append(arrs)

    return structure, per_core_arrays


def _build(structure):
    slots = structure["slots"]
    seg_start = structure["seg_start"]
    CA, CB = structure["CA"], structure["CB"]
    NR = -(-NBLK // GG)  # gather groups per window
    ATILES = -(-CA // 16)
    BTILES = -(-CB // 16)

    nc = bacc.Bacc(None, target_bir_lowering=False, debug=True, num_swdge_queues=4)

    emb16 = nc.dram_tensor("emb16", [V, F], BF16, kind="ExternalInput")
    wst = nc.dram_tensor("wst", [F, H], BF16, kind="ExternalInput")  # W_self.T
    wnt = nc.dram_tensor("wnt", [F, H], BF16, kind="ExternalInput")  # W_neigh.T/32
    brow_d = nc.dram_tensor("brow", [128, 2 * H], BF16, kind="ExternalInput")
    eone_d = nc.dram_tensor("eone", [128, 128], BF16, kind="ExternalInput")
    iota16_d = nc.dram_tensor("iota16", [128, 16 * 128], BF16, kind="ExternalInput")
    tagsA_d = nc.dram_tensor("tagsA", [128, CA], BF16, kind="ExternalInput")
    tagsB_d = nc.dram_tensor("tagsB", [128, CB], BF16, kind="ExternalInput")
    idx_d = [
        [
            nc.dram_tensor(
                f"idx{q}_{r}",
                [
                    128,
                    8
                    * int(
                        seg_start[q][min(NBLK, GG * (r + 1))] - seg_start[q][GG * r]
                    ),
                ],
                I16,
                kind="ExternalInput",
            )
            for r in range(NR)
        ]
        for q in range(NQ)
    ]
    out = nc.dram_tensor("out", [BPC, 2 * H], F32, kind="ExternalOutput")

    SMAX = [
        max(
            int(seg_start[q][min(NBLK, GG * (r + 1))] - seg_start[q][GG * r])
            for r in range(NR)
        )
        for q in range(NQ)
    ]

    with tile.TileContext(nc) as tc, ExitStack() as ctx:
        const = ctx.enter_context(tc.tile_pool(name="const", bufs=1))
        GBUFS = int(os.environ.get("KGBUFS", "3"))
        gpools = [
            ctx.enter_context(tc.tile_pool(name=f"g{q}", bufs=GBUFS))
            for q in range(NQ)
        ]
        apool = ctx.enter_context(tc.tile_pool(name="a", bufs=6))
        bpool = ctx.enter_context(tc.tile_pool(name="b", bufs=2))
        sumpool = ctx.enter_context(tc.tile_pool(name="sum", bufs=4))
        opool = ctx.enter_context(tc.tile_pool(name="ostage", bufs=2))
        ps_sel = ctx.enter_context(tc.tile_pool(name="ps_sel", bufs=2, space="PSUM"))
        ps_out = ctx.enter_context(tc.tile_pool(name="ps_out", bufs=2, space="PSUM"))

        nc.gpsimd.load_library(library_config.mlp)

        wt = {}
        for path, dram in (("s", wst), ("n", wnt)):
            for k in range(2):
                t = const.tile([128, H], BF16, tag=f"w{path}{k}")
                nc.sync.dma_start(out=t[:], in_=dram[128 * k : 128 * (k + 1), :])
                wt[path, k] = t
        brow_t = const.tile([128, 2 * H], BF16)
        nc.sync.dma_start(out=brow_t[:], in_=brow_d[:])
        eone_t = const.tile([128, 128], BF16)
        nc.sync.dma_start(out=eone_t[:], in_=eone_d[:])
        iota16_t = const.tile([128, 16 * 128], BF16)
        nc.sync.dma_start(out=iota16_t[:], in_=iota16_d[:])
        iota16_3d = iota16_t[:].rearrange("p (a b) -> p a b", b=128)
        tagsA_t = const.tile([128, CA], BF16)
        nc.sync.dma_start(out=tagsA_t[:], in_=tagsA_d[:])
        tagsB_t = const.tile([128, CB], BF16)
        nc.sync.dma_start(out=tagsB_t[:], in_=tagsB_d[:])
        def load_idx(dram, tag):
            t = const.tile([128, dram.shape[1]], I16, tag=tag)
            nc.sync.dma_start(out=t[:], in_=dram[:, :])
            return t

        idx_t = [
            [load_idx(idx_d[q][r], f"idx{q}_{r}") for r in range(NR)]
            for q in range(NQ)
        ]

        nrep = int(os.environ.get("KREPEAT", "1"))  # perf probing only
        qctr = [0]

        def pick_q(q):
            picked = qctr[0] % NQ if KQMODE == "rr" else q
            qctr[0] += 1
            return picked

        for _rep in range(nrep):
            g_tiles = {}

            def emit_gathers(r):
                for q in range(NQ):
                    span = int(
                        seg_start[q][min(NBLK, GG * (r + 1))] - seg_start[q][GG * r]
                    )
                    g = gpools[q].tile([128, SMAX[q], F], BF16, tag=f"g{q}")
                    nc.gpsimd.dma_gather(
                        g[:, 0:span, :],
                        emb16[CW * q :, :],
                        idx_t[q][r][:, :],
                        span * 128,
                        span * 128,
                        F,
                        single_packet=False,
                        queue_num=pick_q(q),
                    )
                    g_tiles[q, r] = g

            # selection matrices, built in 16-column tiles (pool-throttled)
            a_tiles = []
            for u in range(ATILES):
                lo, hi = 16 * u, min(CA, 16 * u + 16)
                at = apool.tile([128, 16, 128], BF16, tag="a")
                nc.vector.tensor_tensor(
                    out=at[:, 0 : hi - lo, :],
                    in0=tagsA_t[:, lo:hi].to_broadcast([128, hi - lo, 128]),
                    in1=iota16_3d[:, 0 : hi - lo, :],
                    op=mybir.AluOpType.is_equal,
                )
                a_tiles.append(at)
            b_tiles = []
            for u in range(BTILES):
                lo, hi = 16 * u, min(CB, 16 * u + 16)
                bt = bpool.tile([128, 16, 128], BF16, tag="b")
                nc.vector.tensor_tensor(
                    out=bt[:, 0 : hi - lo, :],
                    in0=tagsB_t[:, lo:hi].to_broadcast([128, hi - lo, 128]),
                    in1=iota16_3d[:, 0 : hi - lo, :],
                    op=mybir.AluOpType.is_equal,
                )
                b_tiles.append(bt)

            ca = cb = 0
            for blk in range(NBLK):
                if blk % GG == 0:
                    emit_gathers(blk // GG)
                psn = ps_sel.tile([128, 2 * 128], F32, tag="pn")
                pss = ps_sel.tile([128, 2 * 128], F32, tag="ps")
                ncols = int(slots[:, blk].sum())
                done = 0
                for q in range(NQ):
                    r = blk // GG
                    g = g_tiles[q, r]
                    rel = int(seg_start[q][blk] - seg_start[q][GG * r])
                    for s in range(int(slots[q][blk])):
                        acol = a_tiles[ca // 16][:, ca % 16, :]
                        for fh in range(2):
                            # one start/stop per PSUM bank: start marks the
                            # whole 2KB zero region pending-zero, so only the
                            # first matmul into the bank may carry it
                            lhs = g[:, rel + s, 128 * fh : 128 * (fh + 1)]
                            nc.tensor.matmul(
                                out=psn[:, 128 * fh : 128 * (fh + 1)],
                                lhsT=lhs,
                                rhs=acol,
                                start=(done == 0 and fh == 0),
                                stop=(done == ncols - 1 and fh == 1),
                                skip_group_check=True,
                            )
                            if s == 0:
                                nc.tensor.matmul(
                                    out=pss[:, 128 * fh : 128 * (fh + 1)],
                                    lhsT=lhs,
                                    rhs=b_tiles[cb // 16][:, cb % 16, :],
                                    start=(q == 0 and fh == 0),
                                    stop=(q == NQ - 1 and fh == 1),
                                    skip_group_check=True,
                                )
                        ca += 1
                        done += 1
                    cb += 1

                ostage = opool.tile([128, 4 * 128], F32, tag="ostage")
                for pi, (path, ps) in enumerate((("s", pss), ("n", psn))):
                    sumt = sumpool.tile([128, 2 * 128], BF16, tag="sum")
                    # ScalarE copy: DVE is loaded with is_equal builds, ACT idle
                    nc.scalar.activation(
                        out=sumt[:],
                        in_=ps[:],
                        func=mybir.ActivationFunctionType.Copy,
                    )
                    po = ps_out.tile([128, 2 * 128], F32, tag=f"po{path}")
                    for k in range(2):
                        nc.tensor.matmul(
                            out=po[:],
                            lhsT=sumt[:, 128 * k : 128 * (k + 1)],
                            rhs=wt[path, k][:],
                            start=(k == 0),
                            stop=False,
                        )
                    nc.tensor.matmul(
                        out=po[:],
                        lhsT=eone_t[:],
                        rhs=brow_t[:, 256 * pi : 256 * (pi + 1)],
                        start=False,
                        stop=True,
                    )
                    nc.scalar.activation(
                        out=ostage[:, 256 * pi : 256 * (pi + 1)],
                        in_=po[:],
                        func=mybir.ActivationFunctionType.Relu,
                    )
                nc.sync.dma_start(
                    out=out[128 * blk : 128 * (blk + 1), :], in_=ostage[:]
                )
            assert ca == CA and cb == CB

    nc.compile()
    return nc


def _prep_and_build(x0, x1):
    structure, per_core = _host_prep(x0, x1)
    key = (
        structure["slots"].tobytes(),
        structure["CA"],
        os.environ.get("KREPEAT", "1"),
        os.environ.get("KGBUFS", "3"),
        GG,
        KQMODE,
    )
    if _BUILT.get("key") != key:
        _BUILT["nc"] = _build(structure)
        _BUILT["key"] = key
    return _BUILT["nc"], structure, per_core


def make_in_maps(x0, x1, emb, W_self, b_self, W_neigh, b_neigh):
    nc, structure, per_core = _prep_and_build(x0, x1)
    emb16 = np.ascontiguousarray(
        np.asarray(emb, dtype=np.float32).astype(ml_dtypes.bfloat16)
    )
    wstv = np.ascontiguousarray(
        np.asarray(W_self, dtype=np.float32).T.astype(ml_dtypes.bfloat16)
    )
    wntv = np.ascontiguousarray(
        (np.asarray(W_neigh, dtype=np.float32).T / NNEIGH).astype(ml_dtypes.bfloat16)
    )
    brow = np.zeros((128, 2 * H), np.float32)
    brow[0, :H] = np.asarray(b_self, dtype=np.float32)
    brow[0, H:] = np.asarray(b_neigh, dtype=np.float32)
    brow = np.ascontiguousarray(brow.astype(ml_dtypes.bfloat16))
    eone = np.zeros((128, 128), np.float32)
    eone[0, :] = 1.0
    eone = np.ascontiguousarray(eone.astype(ml_dtypes.bfloat16))
    iota16 = np.ascontiguousarray(
        np.tile(np.arange(128, dtype=np.float32), (128, 16)).astype(ml_dtypes.bfloat16)
    )
    NR = -(-NBLK // GG)
    in_maps = []
    for c in range(N_CORES):
        m = {
            "emb16": emb16,
            "wst": wstv,
            "wnt": wntv,
            "brow": brow,
            "eone": eone,
            "iota16": iota16,
            "tagsA": per_core[c]["tagsA"],
            "tagsB": per_core[c]["tagsB"],
        }
        for q in range(NQ):
            for r in range(NR):
                m[f"idx{q}_{r}"] = per_core[c]["idx"][q][r]
        in_maps.append(m)
    return nc, in_maps


def kernel(x0, x1, emb, W_self, b_self, W_neigh, b_neigh, **_ignored):
    nc, in_maps = make_in_maps(x0, x1, emb, W_self, b_self, W_neigh, b_neigh)
    res = run_bass_kernel_spmd(nc, in_maps, core_ids=list(range(N_CORES)))
    return np.concatenate([r["out"] for r in res.results], axis=0)
